# revision 25
# baseline (speedup 1.0000x reference)
# Trainium2 Bass kernel for nn_AxonalConnections (gnn_message_passing).
#
# Computes out[B, H, W] = (spikes.reshape(B, N) @ adjacency.T).reshape(B, H, W)
# with B=16, H=W=128, N=16384 on 8 NeuronCores.
#
# Strategy (pure tensor parallelism, no collectives):
#   - Shard adjacency row-wise (target dim) across 8 cores: core i owns
#     target columns [i*2048, (i+1)*2048) of the output.
#   - Host-side, transpose each shard to [source, target] layout so the
#     contraction dim (source) lands on SBUF partitions with unit-stride DMAs.
#   - The kernel is HBM/DMA-bandwidth bound, so minimize bytes:
#     * adjacency ships as fp16 (values ~N(0, 0.02^2), well inside fp16
#       range; 2^-11 relative representation error -> ~1e-4 output error).
#       fp32 would also stream 4x slower through the PE; fp16 streams at
#       full rate (1 column/cycle).
#     * input-adaptive source pruning: per shard, only source rows between
#       the first and last nonzero 128-row chunk contribute; the host
#       detects that range and ships only it. For conv-structured
#       adjacencies this is ~22/128 chunks; for dense inputs it degrades
#       to the full range and stays correct.
#   - Spikes (tiny) are split into fp16 hi + fp16 lo (exact to ~2^-22) and
#     packed as the stationary operand [spikes_hi | spikes_lo] (32 columns).
#     PSUM accumulates [32, 2048] fp32; rows 0-15 = hi terms, rows 16-31 =
#     lo terms; host folds them and concatenates the target shards.
#
# Single-queue HWDGE DMA with 8 KiB per-partition runs sustains ~410 GB/s
# (95% of the 435 GB/s SBUF-AXI fabric ceiling).

import numpy as np

B = 16
H = 128
W = 128
N = H * W            # 16384 source == target size
NCORES = 8
TSH = N // NCORES    # 2048 target columns per core
P = 128              # SBUF partitions / contraction tile
SCHUNKS = N // P     # 128 source chunks
GROUP = 4            # max source chunks per DMA (GROUP * 0.5 MiB per transfer)
NFREE = 512          # matmul moving free dim (one PSUM bank of fp32)

_cache = {}


def _build_nc(n_chunks):
    """Build + compile the SPMD Bass program for `n_chunks` source chunks."""
    import concourse.mybir as mybir
    import concourse.tile as tile
    from concourse import bacc

    assert n_chunks % 2 == 0 and n_chunks > 0
    # 2 MiB transfers amortize per-DMA issue overhead best (1 MiB streams at
    # ~342 GB/s vs ~410 GB/s); a trailing 1 MiB group avoids shipping padded
    # zero chunks when the live count is not a multiple of GROUP.
    group_sizes = [GROUP] * (n_chunks // GROUP)
    if n_chunks % GROUP:
        group_sizes.append(n_chunks % GROUP)

    nc = bacc.Bacc(
        "TRN2",
        target_bir_lowering=False,
        debug=False,
        num_devices=NCORES,
    )
    # a16: live slice of the transposed fp16 adjacency shard, with two
    # source-chunks packed per DRAM row so every DMA descriptor moves a
    # contiguous 8 KiB run per partition. 8 KiB packets are the measured
    # DMA sweet spot: 4 KiB packets pay ~12 ns/packet overhead (320 GB/s)
    # and 16 KiB packets pipeline worse per engine (322 GB/s); 8 KiB
    # sustains ~410 GB/s.
    #   a16[g2*128 + p, half*TSH + t] =
    #       fp16(adj[t0 + t, (c_lo + 2*g2 + half)*128 + p])
    a16 = nc.dram_tensor(
        "a16", [n_chunks * P // 2, 2 * TSH], mybir.dt.float16, kind="ExternalInput"
    ).ap()
    # spk: stationary weights for the live chunk range, packed
    # [P, n_chunks*32] fp16 where
    #   spk[p, k*32 + b]      = fp16_hi(spikes[b, (c_lo + k)*128 + p])
    #   spk[p, k*32 + 16 + b] = fp16_lo(spikes[b, (c_lo + k)*128 + p])
    spk = nc.dram_tensor(
        "spk", [P, n_chunks * 32], mybir.dt.float16, kind="ExternalInput"
    ).ap()
    out = nc.dram_tensor("o", [32, TSH], mybir.dt.float32, kind="ExternalOutput").ap()

    f32 = mybir.dt.float32
    f16 = mybir.dt.float16
    NJ = TSH // NFREE  # 4 PSUM banks

    with tile.TileContext(nc) as tc:
        with (
            # Enough buffers to prefetch deeply (the HWDGE completion-sem
            # pool caps useful depth at ~8 DMAs anyway). Capped so the
            # dense fallback still fits SBUF.
            tc.tile_pool(name="adj", bufs=min(8, len(group_sizes))) as adj_pool,
            tc.tile_pool(name="spkp", bufs=1) as spk_pool,
            tc.tile_pool(name="psum", bufs=1, space="PSUM") as psum_pool,
            tc.tile_pool(name="outp", bufs=1) as out_pool,
        ):
            # Stationary weights go on the ACT HWDGE ring so the transfer
            # overlaps the first adjacency groups on the SP ring. The
            # adjacency stream itself stays on ONE queue: splitting it
            # across SP and ACT makes each SDMA engine alternate queues per
            # packet, costing ~15% utilization (measured 320 vs 422 GB/s).
            spk_t = spk_pool.tile([P, n_chunks * 32], f16)
            nc.scalar.dma_start(spk_t[:], spk[:])

            # One PSUM tile per output bank so the tail copies don't
            # serialize against the other banks' matmuls (Tile tracks
            # PSUM dependencies at tile granularity).
            ps = [
                psum_pool.tile([32, NFREE], f32, name=f"ps{j}", tag=f"ps{j}")
                for j in range(NJ)
            ]
            ot = out_pool.tile([32, TSH], f32)

            ngroups = len(group_sizes)
            c0_of_group = [sum(group_sizes[:g]) for g in range(ngroups)]
            for g, gsz in enumerate(group_sizes):
                c0g = c0_of_group[g]
                rows = (gsz // 2) * P  # packed DRAM rows in this group
                r0 = c0g * P // 2
                at = adj_pool.tile([P, gsz * TSH], f16, name=f"at{g}", tag="at")
                nc.sync.dma_start(
                    at[:].rearrange("p (n t) -> p n t", n=gsz // 2),
                    a16[r0 : r0 + rows, :].rearrange("(n p) t -> p n t", p=P),
                )
                last_group = g == ngroups - 1
                if not last_group:
                    for nl in range(gsz):
                        n = c0g + nl
                        w = spk_t[:, n * 32 : (n + 1) * 32]
                        for j in range(NJ):
                            c0 = nl * TSH + j * NFREE
                            nc.tensor.matmul(
                                ps[j][:, :],
                                w,
                                at[:, c0 : c0 + NFREE],
                                start=(n == 0),
                                stop=False,
                            )
                else:
                    # Final group: finish one PSUM bank at a time so the
                    # PSUM->SBUF copy and the output DMA overlap the
                    # remaining banks' matmuls instead of serializing after
                    # the last one.
                    for j in range(NJ):
                        sl = slice(j * NFREE, (j + 1) * NFREE)
                        for nl in range(gsz):
                            n = c0g + nl
                            w = spk_t[:, n * 32 : (n + 1) * 32]
                            nc.tensor.matmul(
                                ps[j][:, :],
                                w,
                                at[:, nl * TSH + j * NFREE : nl * TSH + (j + 1) * NFREE],
                                start=(n == 0),
                                stop=(nl == gsz - 1),
                            )
                        nc.vector.tensor_copy(ot[:, sl], ps[j][:, :])
                        nc.sync.dma_start(out[:, sl], ot[:, sl])

    nc.compile()
    return nc


def _get_nc(n_chunks):
    key = ("nc", n_chunks)
    if key not in _cache:
        _cache[key] = _build_nc(n_chunks)
    return _cache[key]


def _split_hi_lo(x32):
    """Split fp32 array into (hi, lo) fp16 parts with x32 ~= hi + lo."""
    hi = x32.astype(np.float16)
    lo = (x32 - hi.astype(np.float32)).astype(np.float16)
    return hi, lo


def _prep_inputs(spikes, adjacency):
    flat = np.ascontiguousarray(np.asarray(spikes, dtype=np.float32).reshape(B, N))
    adj = np.asarray(adjacency, dtype=np.float32)

    # Live source-chunk range per target shard: chunk c contributes to core
    # i's outputs only if adj[i*TSH:(i+1)*TSH, c*128:(c+1)*128] has any
    # nonzero. Shipping [first_live, last_live] keeps the kernel exact for
    # every input while skipping the all-zero bands of conv-structured
    # adjacencies.
    blocks = np.any(
        adj.reshape(NCORES, TSH, SCHUNKS, P) != 0.0, axis=(1, 3)
    )  # [NCORES, SCHUNKS]
    c_lo = np.zeros(NCORES, np.int64)
    c_len = np.full(NCORES, SCHUNKS, np.int64)
    for i in range(NCORES):
        nz = np.nonzero(blocks[i])[0]
        if len(nz):
            c_lo[i], c_len[i] = nz[0], nz[-1] - nz[0] + 1
        else:
            c_lo[i], c_len[i] = 0, 1
    n_chunks = int(min(SCHUNKS, -(-int(c_len.max()) // 2) * 2))
    # Clamp each core's range to [0, SCHUNKS - n_chunks].
    c_lo = np.minimum(c_lo, SCHUNKS - n_chunks)

    flatT = np.ascontiguousarray(flat.T)  # [N, B]
    fhi, flo = _split_hi_lo(flatT)
    spk_full = np.empty((SCHUNKS, P, 32), np.float16)  # [c, p, 2*B]
    spk_full[:, :, :B] = fhi.reshape(SCHUNKS, P, B)
    spk_full[:, :, B:] = flo.reshape(SCHUNKS, P, B)

    adjT = adj.T  # [source, target] view (strided)
    in_maps = []
    for i in range(NCORES):
        lo = int(c_lo[i])
        s0, s1 = lo * P, (lo + n_chunks) * P
        a16 = adjT[s0:s1, i * TSH : (i + 1) * TSH].astype(np.float16)
        # Pack two source-chunks per DRAM row (see kernel comment).
        a16 = np.ascontiguousarray(
            a16.reshape(n_chunks // 2, 2, P, TSH).transpose(0, 2, 1, 3)
        ).reshape(n_chunks * P // 2, 2 * TSH)
        spk = np.ascontiguousarray(
            spk_full[lo : lo + n_chunks].transpose(1, 0, 2)
        ).reshape(P, n_chunks * 32)
        in_maps.append({"a16": a16, "spk": spk})
    return n_chunks, in_maps


def _run(n_chunks, in_maps, **kwargs):
    from concourse.bass_utils import run_bass_kernel_spmd

    return run_bass_kernel_spmd(
        _get_nc(n_chunks), in_maps, core_ids=list(range(NCORES)), **kwargs
    )


def kernel(spikes, adjacency):
    n_chunks, in_maps = _prep_inputs(spikes, adjacency)
    res = _run(n_chunks, in_maps)
    outs = [r["o"] for r in res.results]
    # Fold hi-weight rows (0:16) + lo-weight rows (16:32), concat target shards.
    full = np.concatenate([o[:B] + o[B:] for o in outs], axis=1)  # [B, N]
    return np.ascontiguousarray(full.reshape(B, H, W), dtype=np.float32)


# revision 26
# speedup vs baseline: 1.6503x; 1.6503x over previous
# Trainium2 Bass kernel for nn_AxonalConnections (gnn_message_passing).
#
# Computes out[B, H, W] = (spikes.reshape(B, N) @ adjacency.T).reshape(B, H, W)
# with B=16, H=W=128, N=16384 on 8 NeuronCores.
#
# Strategy (pure tensor parallelism, no collectives):
#   - Shard adjacency row-wise (target dim) across 8 cores: core i owns
#     target columns [i*2048, (i+1)*2048) of the output.
#   - The kernel is HBM/DMA-bandwidth bound, so minimize shipped bytes:
#     * adjacency ships as fp16 (values ~N(0, 0.02^2), well inside fp16
#       range; 2^-11 relative representation error -> ~1e-4 output error).
#       fp32 would also stream 4x slower through the PE; fp16 streams at
#       full rate (1 column/cycle).
#     * input-adaptive block pruning: the host scans the adjacency at
#       [128 x 128] block granularity (source grid-row si x target grid-row
#       ti) and ships only blocks that contain nonzeros. For the conv-
#       structured adjacency this is ~112 of 2048 blocks per core (3.7 MiB
#       vs 64 MiB); for dense inputs every block ships and the kernel stays
#       exact. Per-core block sets are aligned by a per-core source offset
#       into one shared pattern so all 8 cores run the same NEFF.
#   - Spikes (tiny) are split into fp16 hi + fp16 lo (exact to ~2^-22) and
#     packed as the stationary operand [spikes_hi | spikes_lo] (32 columns).
#     PSUM accumulates [32, 128]-per-block into 4 banks of [32, 512]; rows
#     0-15 = hi terms, rows 16-31 = lo terms; host folds and concatenates.
#   - Blocks stream ti-major, so each PSUM bank finishes early and its
#     PSUM->SBUF copy + output DMA overlap the remaining matmuls.
#
# Single-queue HWDGE DMA with 8 KiB per-partition runs sustains ~410 GB/s
# (95% of the 435 GB/s SBUF-AXI fabric ceiling).

import numpy as np

B = 16
H = 128
W = 128
N = H * W            # 16384 source == target size
NCORES = 8
TSH = N // NCORES    # 2048 target columns per core
TI = TSH // W        # 16 target grid-rows per core
P = 128              # SBUF partitions / contraction tile
SCHUNKS = N // P     # 128 source chunks (== source grid-rows)
BLK_GROUP = 32       # blocks per DMA (32 * 32 KiB = 1 MiB, 8 KiB runs)
BLK = P * P          # elements per block

_cache = {}


def _build_nc(pattern, n_spk):
    """Build + compile the SPMD Bass program.

    pattern: sorted list of (ti, si_rel) block coordinates, ti-major,
             identical for all cores. Every ti in [0, TI) appears.
    n_spk:   number of stationary source chunks shipped (max si_rel + 1).
    """
    import concourse.mybir as mybir
    import concourse.tile as tile
    from concourse import bacc

    n_blocks = len(pattern)
    group_sizes = [BLK_GROUP] * (n_blocks // BLK_GROUP)
    if n_blocks % BLK_GROUP:
        group_sizes.append(n_blocks % BLK_GROUP)

    nc = bacc.Bacc(
        "TRN2",
        target_bir_lowering=False,
        debug=False,
        num_devices=NCORES,
    )
    # ablk: flat stream of gathered [128 x 128] fp16 blocks in `pattern`
    # order, packed per DMA-group as [p, group_blocks*128] (partition-major)
    # so every descriptor moves one contiguous 8 KiB run per partition.
    ablk = nc.dram_tensor(
        "ablk", [n_blocks * BLK], mybir.dt.float16, kind="ExternalInput"
    ).ap()
    # spk: stationary weights for the shipped source-chunk window, packed
    # [P, n_spk*32] fp16 where
    #   spk[p, k*32 + b]      = fp16_hi(spikes[b, (o_i + k)*128 + p])
    #   spk[p, k*32 + 16 + b] = fp16_lo(spikes[b, (o_i + k)*128 + p])
    # (o_i = per-core source offset; out-of-range chunks are zero).
    spk = nc.dram_tensor(
        "spk", [P, n_spk * 32], mybir.dt.float16, kind="ExternalInput"
    ).ap()
    out = nc.dram_tensor("o", [32, TSH], mybir.dt.float32, kind="ExternalOutput").ap()

    f32 = mybir.dt.float32
    f16 = mybir.dt.float16
    NJ = 4  # PSUM banks ([32, 512] each; 4 ti-blocks per bank)

    # Per-ti first/last stream index (pattern is ti-major sorted).
    first_k = {}
    last_k = {}
    for k, (ti, _) in enumerate(pattern):
        first_k.setdefault(ti, k)
        last_k[ti] = k

    with tile.TileContext(nc) as tc:
        with (
            tc.tile_pool(name="adj", bufs=min(8, len(group_sizes))) as adj_pool,
            tc.tile_pool(name="spkp", bufs=1) as spk_pool,
            tc.tile_pool(name="psum", bufs=1, space="PSUM") as psum_pool,
            tc.tile_pool(name="outp", bufs=1) as out_pool,
        ):
            # Stationary weights on the ACT HWDGE ring so the transfer
            # overlaps the first block groups on the SP ring. The block
            # stream itself stays on ONE queue: splitting it across SP and
            # ACT makes each SDMA engine alternate queues per packet,
            # costing ~15% utilization (measured 320 vs 422 GB/s).
            spk_t = spk_pool.tile([P, n_spk * 32], f16)
            nc.scalar.dma_start(spk_t[:], spk[:])

            ps = [
                psum_pool.tile([32, NJ * P], f32, name=f"ps{j}", tag=f"ps{j}")
                for j in range(NJ)
            ]
            ot = out_pool.tile([32, TSH], f32)

            k = 0
            off = 0
            for g, gsz in enumerate(group_sizes):
                at = adj_pool.tile([P, gsz * P], f16, name=f"at{g}", tag="at")
                nc.sync.dma_start(
                    at[:].rearrange("p (n t) -> p n t", n=gsz),
                    ablk[off : off + gsz * BLK].rearrange("(p n t) -> p n t", p=P, t=P),
                )
                off += gsz * BLK
                for kl in range(gsz):
                    ti, si_rel = pattern[k]
                    j, c = divmod(ti, NJ)
                    nc.tensor.matmul(
                        ps[j][:, c * P : (c + 1) * P],
                        spk_t[:, si_rel * 32 : (si_rel + 1) * 32],
                        at[:, kl * P : (kl + 1) * P],
                        start=(k == first_k[ti]),
                        stop=(k == last_k[ti]),
                    )
                    if k == last_k[ti] and ti % NJ == NJ - 1:
                        # Bank j fully accumulated: drain it while the
                        # remaining banks' matmuls keep streaming.
                        sl = slice(j * NJ * P, (j + 1) * NJ * P)
                        nc.vector.tensor_copy(ot[:, sl], ps[j][:, :])
                        nc.sync.dma_start(out[:, sl], ot[:, sl])
                    k += 1

    nc.compile()
    return nc


def _get_nc(pattern, n_spk):
    key = (tuple(pattern), n_spk)
    if key not in _cache:
        _cache[key] = _build_nc(pattern, n_spk)
    return _cache[key]


def _split_hi_lo(x32):
    """Split fp32 array into (hi, lo) fp16 parts with x32 ~= hi + lo."""
    hi = x32.astype(np.float16)
    lo = (x32 - hi.astype(np.float32)).astype(np.float16)
    return hi, lo


def _prep_inputs(spikes, adjacency):
    flat = np.ascontiguousarray(np.asarray(spikes, dtype=np.float32).reshape(B, N))
    adj = np.asarray(adjacency, dtype=np.float32)

    # Live [ti, si] block map per core: block contributes to core i's
    # outputs iff adj[i*TSH + ti*128 : .. + 128, si*128 : (si+1)*128] has a
    # nonzero. Shipping exactly the live blocks keeps the kernel exact for
    # every input while skipping the zero blocks of conv-structured
    # adjacencies.
    bm = np.any(
        adj.reshape(NCORES, TI, W, SCHUNKS, P) != 0.0, axis=(2, 4)
    )  # [core, ti, si]

    # Align per-core block sets into one shared pattern via a per-core
    # source offset o_i (cores run one SPMD program). o_i = min(si - ti)
    # over live blocks aligns banded structures exactly.
    offs = np.zeros(NCORES, np.int64)
    pat = set()
    for i in range(NCORES):
        tis, sis = np.nonzero(bm[i])
        offs[i] = (sis - tis).min() if len(tis) else 0
        pat.update(zip(tis.tolist(), (sis - offs[i]).tolist()))
    for ti in range(TI):  # every ti needs >=1 block so PSUM gets initialized
        if not any(t == ti for t, _ in pat):
            pat.add((ti, 0))
    pattern = sorted(pat)
    n_spk = max(s for _, s in pattern) + 1

    # Stationary weights (hi/lo split), indexed by absolute source chunk.
    flatT = np.ascontiguousarray(flat.T)  # [N, B]
    fhi, flo = _split_hi_lo(flatT)
    spk_full = np.zeros((SCHUNKS, P, 32), np.float16)  # [si, p, 2*B]
    spk_full[:, :, :B] = fhi.reshape(SCHUNKS, P, B)
    spk_full[:, :, B:] = flo.reshape(SCHUNKS, P, B)

    n_blocks = len(pattern)
    group_sizes = [BLK_GROUP] * (n_blocks // BLK_GROUP)
    if n_blocks % BLK_GROUP:
        group_sizes.append(n_blocks % BLK_GROUP)

    in_maps = []
    for i in range(NCORES):
        o = int(offs[i])
        t0 = i * TSH
        blocks = np.zeros((n_blocks, P, P), np.float16)  # [k, sj, tj]
        for k, (ti, si_rel) in enumerate(pattern):
            si = o + si_rel
            if 0 <= si < SCHUNKS and bm[i, ti, si]:
                blocks[k] = (
                    adj[t0 + ti * W : t0 + (ti + 1) * W, si * P : (si + 1) * P]
                    .T.astype(np.float16)
                )
        # Pack each DMA group partition-major: [p, gsz*128].
        parts = []
        k0 = 0
        for gsz in group_sizes:
            parts.append(
                np.ascontiguousarray(blocks[k0 : k0 + gsz].transpose(1, 0, 2)).ravel()
            )
            k0 += gsz
        ablk = np.concatenate(parts)

        spk = np.zeros((n_spk, P, 32), np.float16)
        s_lo = max(0, -o)
        s_hi = min(n_spk, SCHUNKS - o)
        if s_hi > s_lo:
            spk[s_lo:s_hi] = spk_full[o + s_lo : o + s_hi]
        spk = np.ascontiguousarray(spk.transpose(1, 0, 2)).reshape(P, n_spk * 32)
        in_maps.append({"ablk": ablk, "spk": spk})
    return pattern, n_spk, in_maps


def _run(pattern, n_spk, in_maps, **kwargs):
    from concourse.bass_utils import run_bass_kernel_spmd

    return run_bass_kernel_spmd(
        _get_nc(pattern, n_spk), in_maps, core_ids=list(range(NCORES)), **kwargs
    )


def kernel(spikes, adjacency):
    pattern, n_spk, in_maps = _prep_inputs(spikes, adjacency)
    res = _run(pattern, n_spk, in_maps)
    outs = [r["o"] for r in res.results]
    # Fold hi-weight rows (0:16) + lo-weight rows (16:32), concat target shards.
    full = np.concatenate([o[:B] + o[B:] for o in outs], axis=1)  # [B, N]
    return np.ascontiguousarray(full.reshape(B, H, W), dtype=np.float32)


# revision 27
# speedup vs baseline: 1.7993x; 1.0903x over previous
# Trainium2 Bass kernel for nn_AxonalConnections (gnn_message_passing).
#
# Computes out[B, H, W] = (spikes.reshape(B, N) @ adjacency.T).reshape(B, H, W)
# with B=16, H=W=128, N=16384 on 8 NeuronCores.
#
# Strategy (pure tensor parallelism, no collectives):
#   - Shard adjacency row-wise (target dim) across 8 cores: core i owns
#     target columns [i*2048, (i+1)*2048) of the output.
#   - The kernel is HBM/DMA-bandwidth bound, so minimize shipped bytes:
#     * adjacency ships as fp16 (values ~N(0, 0.02^2), well inside fp16
#       range; 2^-11 relative representation error -> ~1e-4 output error).
#       fp32 would also stream 4x slower through the PE; fp16 streams at
#       full rate (1 column/cycle).
#     * input-adaptive block pruning: the host scans the adjacency at
#       [128 x 128] block granularity (source grid-row si x target grid-row
#       ti) and ships only blocks that contain nonzeros. For the conv-
#       structured adjacency this is ~112 of 2048 blocks per core (3.7 MiB
#       vs 64 MiB); for dense inputs every block ships and the kernel stays
#       exact. Per-core block sets are aligned by a per-core source offset
#       into one shared pattern so all 8 cores run the same NEFF.
#   - Spikes (tiny) are split into fp16 hi + fp16 lo (exact to ~2^-22) and
#     packed as the stationary operand [spikes_hi | spikes_lo] (32 columns).
#     PSUM accumulates [32, 128]-per-block into 4 banks of [32, 512]; rows
#     0-15 = hi terms, rows 16-31 = lo terms; host folds and concatenates.
#   - Blocks stream ti-major, so each PSUM bank finishes early and its
#     PSUM->SBUF copy + output DMA overlap the remaining matmuls.
#
# Single-queue HWDGE DMA with 8 KiB per-partition runs sustains ~410 GB/s
# (95% of the 435 GB/s SBUF-AXI fabric ceiling).

import numpy as np

B = 16
H = 128
W = 128
N = H * W            # 16384 source == target size
NCORES = 8
TSH = N // NCORES    # 2048 target columns per core
TI = TSH // W        # 16 target grid-rows per core
P = 128              # SBUF partitions / contraction tile
SCHUNKS = N // P     # 128 source chunks (== source grid-rows)
BLK_GROUP = 32       # blocks per DMA (32 * 32 KiB = 1 MiB, 8 KiB runs)
BLK = P * P          # elements per block

_cache = {}


def _build_nc(pattern, n_spk):
    """Build + compile the SPMD Bass program.

    pattern: sorted list of (ti, si_rel) block coordinates, ti-major,
             identical for all cores. Every ti in [0, TI) appears.
    n_spk:   number of stationary source chunks shipped (max si_rel + 1).
    """
    import concourse.mybir as mybir
    import concourse.tile as tile
    from concourse import bacc

    n_blocks = len(pattern)
    group_sizes = [BLK_GROUP] * (n_blocks // BLK_GROUP)
    if n_blocks % BLK_GROUP:
        group_sizes.append(n_blocks % BLK_GROUP)

    nc = bacc.Bacc(
        "TRN2",
        target_bir_lowering=False,
        debug=False,
        num_devices=NCORES,
    )
    # ablk: flat stream of gathered [128 x 128] fp16 blocks in `pattern`
    # order, packed per DMA-group as [p, group_blocks*128] (partition-major)
    # so every descriptor moves one contiguous 8 KiB run per partition.
    ablk = nc.dram_tensor(
        "ablk", [n_blocks * BLK], mybir.dt.float16, kind="ExternalInput"
    ).ap()
    # spk: stationary weights for the shipped source-chunk window, packed
    # [P, n_spk*32] fp16 where
    #   spk[p, k*32 + b]      = fp16_hi(spikes[b, (o_i + k)*128 + p])
    #   spk[p, k*32 + 16 + b] = fp16_lo(spikes[b, (o_i + k)*128 + p])
    # (o_i = per-core source offset; out-of-range chunks are zero).
    spk = nc.dram_tensor(
        "spk", [P, n_spk * 32], mybir.dt.float16, kind="ExternalInput"
    ).ap()
    out = nc.dram_tensor("o", [32, TSH], mybir.dt.float32, kind="ExternalOutput").ap()

    f32 = mybir.dt.float32
    f16 = mybir.dt.float16
    NJ = 4  # PSUM banks ([32, 512] each; 4 ti-blocks per bank)

    # Per-ti first/last stream index (pattern is ti-major sorted).
    first_k = {}
    last_k = {}
    for k, (ti, _) in enumerate(pattern):
        first_k.setdefault(ti, k)
        last_k[ti] = k

    with tile.TileContext(nc) as tc:
        with (
            tc.tile_pool(name="adj", bufs=min(8, len(group_sizes))) as adj_pool,
            tc.tile_pool(name="spkp", bufs=1) as spk_pool,
            tc.tile_pool(name="psum", bufs=1, space="PSUM") as psum_pool,
            tc.tile_pool(name="outp", bufs=1) as out_pool,
        ):
            # Stationary weights on the ACT HWDGE ring so the transfer
            # overlaps the first block groups on the SP ring. The block
            # stream itself stays on ONE queue: splitting it across SP and
            # ACT makes each SDMA engine alternate queues per packet,
            # costing ~15% utilization (measured 320 vs 422 GB/s).
            spk_t = spk_pool.tile([P, n_spk * 32], f16)
            nc.scalar.dma_start(spk_t[:], spk[:])

            ps = [
                psum_pool.tile([32, NJ * P], f32, name=f"ps{j}", tag=f"ps{j}")
                for j in range(NJ)
            ]
            ot = out_pool.tile([32, TSH], f32)

            k = 0
            off = 0
            for g, gsz in enumerate(group_sizes):
                at = adj_pool.tile([P, gsz * P], f16, name=f"at{g}", tag="at")
                nc.sync.dma_start(
                    at[:].rearrange("p (n t) -> p n t", n=gsz),
                    ablk[off : off + gsz * BLK].rearrange("(p n t) -> p n t", p=P, t=P),
                )
                off += gsz * BLK
                for kl in range(gsz):
                    ti, si_rel = pattern[k]
                    j, c = divmod(ti, NJ)
                    nc.tensor.matmul(
                        ps[j][:, c * P : (c + 1) * P],
                        spk_t[:, si_rel * 32 : (si_rel + 1) * 32],
                        at[:, kl * P : (kl + 1) * P],
                        start=(k == first_k[ti]),
                        stop=(k == last_k[ti]),
                    )
                    if k == last_k[ti] and ti % NJ == NJ - 1:
                        # Bank j fully accumulated: drain it while the
                        # remaining banks' matmuls keep streaming. The store
                        # goes on the ACT HWDGE ring — on the (in-order) SP
                        # ring its semaphore wait would block later block-
                        # group DMA issues behind it.
                        sl = slice(j * NJ * P, (j + 1) * NJ * P)
                        nc.vector.tensor_copy(ot[:, sl], ps[j][:, :])
                        nc.scalar.dma_start(out[:, sl], ot[:, sl])
                    k += 1

    nc.compile()
    return nc


def _get_nc(pattern, n_spk):
    key = (tuple(pattern), n_spk)
    if key not in _cache:
        _cache[key] = _build_nc(pattern, n_spk)
    return _cache[key]


def _split_hi_lo(x32):
    """Split fp32 array into (hi, lo) fp16 parts with x32 ~= hi + lo."""
    hi = x32.astype(np.float16)
    lo = (x32 - hi.astype(np.float32)).astype(np.float16)
    return hi, lo


def _prep_inputs(spikes, adjacency):
    flat = np.ascontiguousarray(np.asarray(spikes, dtype=np.float32).reshape(B, N))
    adj = np.asarray(adjacency, dtype=np.float32)

    # Live [ti, si] block map per core: block contributes to core i's
    # outputs iff adj[i*TSH + ti*128 : .. + 128, si*128 : (si+1)*128] has a
    # nonzero. Shipping exactly the live blocks keeps the kernel exact for
    # every input while skipping the zero blocks of conv-structured
    # adjacencies.
    bm = np.any(
        adj.reshape(NCORES, TI, W, SCHUNKS, P) != 0.0, axis=(2, 4)
    )  # [core, ti, si]

    # Align per-core block sets into one shared pattern via a per-core
    # source offset o_i (cores run one SPMD program). o_i = min(si - ti)
    # over live blocks aligns banded structures exactly.
    offs = np.zeros(NCORES, np.int64)
    pat = set()
    for i in range(NCORES):
        tis, sis = np.nonzero(bm[i])
        offs[i] = (sis - tis).min() if len(tis) else 0
        pat.update(zip(tis.tolist(), (sis - offs[i]).tolist()))
    for ti in range(TI):  # every ti needs >=1 block so PSUM gets initialized
        if not any(t == ti for t, _ in pat):
            pat.add((ti, 0))
    pattern = sorted(pat)
    n_spk = max(s for _, s in pattern) + 1

    # Stationary weights (hi/lo split), indexed by absolute source chunk.
    flatT = np.ascontiguousarray(flat.T)  # [N, B]
    fhi, flo = _split_hi_lo(flatT)
    spk_full = np.zeros((SCHUNKS, P, 32), np.float16)  # [si, p, 2*B]
    spk_full[:, :, :B] = fhi.reshape(SCHUNKS, P, B)
    spk_full[:, :, B:] = flo.reshape(SCHUNKS, P, B)

    n_blocks = len(pattern)
    group_sizes = [BLK_GROUP] * (n_blocks // BLK_GROUP)
    if n_blocks % BLK_GROUP:
        group_sizes.append(n_blocks % BLK_GROUP)

    in_maps = []
    for i in range(NCORES):
        o = int(offs[i])
        t0 = i * TSH
        blocks = np.zeros((n_blocks, P, P), np.float16)  # [k, sj, tj]
        for k, (ti, si_rel) in enumerate(pattern):
            si = o + si_rel
            if 0 <= si < SCHUNKS and bm[i, ti, si]:
                blocks[k] = (
                    adj[t0 + ti * W : t0 + (ti + 1) * W, si * P : (si + 1) * P]
                    .T.astype(np.float16)
                )
        # Pack each DMA group partition-major: [p, gsz*128].
        parts = []
        k0 = 0
        for gsz in group_sizes:
            parts.append(
                np.ascontiguousarray(blocks[k0 : k0 + gsz].transpose(1, 0, 2)).ravel()
            )
            k0 += gsz
        ablk = np.concatenate(parts)

        spk = np.zeros((n_spk, P, 32), np.float16)
        s_lo = max(0, -o)
        s_hi = min(n_spk, SCHUNKS - o)
        if s_hi > s_lo:
            spk[s_lo:s_hi] = spk_full[o + s_lo : o + s_hi]
        spk = np.ascontiguousarray(spk.transpose(1, 0, 2)).reshape(P, n_spk * 32)
        in_maps.append({"ablk": ablk, "spk": spk})
    return pattern, n_spk, in_maps


def _run(pattern, n_spk, in_maps, **kwargs):
    from concourse.bass_utils import run_bass_kernel_spmd

    return run_bass_kernel_spmd(
        _get_nc(pattern, n_spk), in_maps, core_ids=list(range(NCORES)), **kwargs
    )


def kernel(spikes, adjacency):
    pattern, n_spk, in_maps = _prep_inputs(spikes, adjacency)
    res = _run(pattern, n_spk, in_maps)
    outs = [r["o"] for r in res.results]
    # Fold hi-weight rows (0:16) + lo-weight rows (16:32), concat target shards.
    full = np.concatenate([o[:B] + o[B:] for o in outs], axis=1)  # [B, N]
    return np.ascontiguousarray(full.reshape(B, H, W), dtype=np.float32)


# revision 32
# speedup vs baseline: 1.8898x; 1.0503x over previous
# Trainium2 Bass kernel for nn_AxonalConnections (gnn_message_passing).
#
# Computes out[B, H, W] = (spikes.reshape(B, N) @ adjacency.T).reshape(B, H, W)
# with B=16, H=W=128, N=16384 on 8 NeuronCores.
#
# Strategy (pure tensor parallelism, no collectives):
#   - Shard adjacency row-wise (target dim) across 8 cores: core i owns
#     target columns [i*2048, (i+1)*2048) of the output.
#   - The kernel is HBM/DMA-bandwidth bound, so minimize shipped bytes:
#     * adjacency ships as fp16 (values ~N(0, 0.02^2), well inside fp16
#       range; 2^-11 relative representation error -> ~1e-4 output error).
#       fp32 would also stream 4x slower through the PE; fp16 streams at
#       full rate (1 column/cycle).
#     * input-adaptive block pruning: the host scans the adjacency at
#       [128 x 128] block granularity (source grid-row si x target grid-row
#       ti) and ships only blocks that contain nonzeros. For the conv-
#       structured adjacency this is ~112 of 2048 blocks per core (3.7 MiB
#       vs 64 MiB); for dense inputs every block ships and the kernel stays
#       exact. Per-core block sets are aligned by a per-core source offset
#       into one shared pattern so all 8 cores run the same NEFF.
#   - Spikes (tiny) are split into fp16 hi + fp16 lo (exact to ~2^-22) and
#     packed as the stationary operand [spikes_hi | spikes_lo] (32 columns).
#     PSUM accumulates [32, 128]-per-block into 4 banks of [32, 512]; rows
#     0-15 = hi terms, rows 16-31 = lo terms; host folds and concatenates.
#   - Blocks stream ti-major, so each PSUM bank finishes early and its
#     PSUM->SBUF copy + output DMA overlap the remaining matmuls.
#
# Single-queue HWDGE DMA with 8 KiB per-partition runs sustains ~410 GB/s
# (95% of the 435 GB/s SBUF-AXI fabric ceiling).

import numpy as np

B = 16
H = 128
W = 128
N = H * W            # 16384 source == target size
NCORES = 8
TSH = N // NCORES    # 2048 target columns per core
TI = TSH // W        # 16 target grid-rows per core
P = 128              # SBUF partitions / contraction tile
SCHUNKS = N // P     # 128 source chunks (== source grid-rows)
BLK_GROUP = 32       # blocks per DMA (32 * 32 KiB = 1 MiB, 8 KiB runs)
BLK = P * P          # elements per block

_cache = {}


def _build_nc(pattern, n_spk):
    """Build + compile the SPMD Bass program.

    pattern: sorted list of (ti, si_rel) block coordinates, ti-major,
             identical for all cores. Every ti in [0, TI) appears.
    n_spk:   number of stationary source chunks shipped (max si_rel + 1).
    """
    import concourse.mybir as mybir
    import concourse.tile as tile
    from concourse import bacc

    n_blocks = len(pattern)
    group_sizes = _group_sizes(n_blocks)

    nc = bacc.Bacc(
        "TRN2",
        target_bir_lowering=False,
        debug=False,
        num_devices=NCORES,
    )
    # ablk: flat stream of gathered [128 x 128] fp16 blocks in `pattern`
    # order, packed per DMA-group as [p, group_blocks*128] (partition-major)
    # so every descriptor moves one contiguous 8 KiB run per partition.
    ablk = nc.dram_tensor(
        "ablk", [n_blocks * BLK], mybir.dt.float16, kind="ExternalInput"
    ).ap()
    # spk: stationary weights for the shipped source-chunk window, packed
    # [P, n_spk*32] fp16 where
    #   spk[p, k*32 + b]      = fp16_hi(spikes[b, (o_i + k)*128 + p])
    #   spk[p, k*32 + 16 + b] = fp16_lo(spikes[b, (o_i + k)*128 + p])
    # (o_i = per-core source offset; out-of-range chunks are zero).
    spk = nc.dram_tensor(
        "spk", [P, n_spk * 32], mybir.dt.float16, kind="ExternalInput"
    ).ap()
    out = nc.dram_tensor("o", [32, TSH], mybir.dt.float32, kind="ExternalOutput").ap()

    f32 = mybir.dt.float32
    f16 = mybir.dt.float16
    NJ = 4  # PSUM banks ([32, 512] each; 4 ti-blocks per bank)

    # Per-ti first/last stream index (pattern is ti-major sorted).
    first_k = {}
    last_k = {}
    for k, (ti, _) in enumerate(pattern):
        first_k.setdefault(ti, k)
        last_k[ti] = k

    with tile.TileContext(nc) as tc:
        with (
            tc.tile_pool(name="adj", bufs=min(8, len(group_sizes))) as adj_pool,
            tc.tile_pool(name="spkp", bufs=1) as spk_pool,
            tc.tile_pool(name="psum", bufs=1, space="PSUM") as psum_pool,
            tc.tile_pool(name="outp", bufs=1) as out_pool,
        ):
            # Stationary weights load first on the SP ring: every matmul
            # waits on them, and on the ACT ring their packets get
            # interleaved behind the block stream (first matmul slips by
            # ~2.5 us). Serializing ~0.5 us ahead of the stream is cheaper.
            spk_t = spk_pool.tile([P, n_spk * 32], f16)
            nc.sync.dma_start(spk_t[:], spk[:])

            ps = [
                psum_pool.tile([32, NJ * P], f32, name=f"ps{j}", tag=f"ps{j}")
                for j in range(NJ)
            ]
            ot = out_pool.tile([32, TSH], f32)

            k = 0
            off = 0
            for g, gsz in enumerate(group_sizes):
                at = adj_pool.tile([P, gsz * P], f16, name=f"at{g}", tag="at")
                nc.sync.dma_start(
                    at[:].rearrange("p (n t) -> p n t", n=gsz),
                    ablk[off : off + gsz * BLK].rearrange("(p n t) -> p n t", p=P, t=P),
                )
                off += gsz * BLK
                for kl in range(gsz):
                    ti, si_rel = pattern[k]
                    j, c = divmod(ti, NJ)
                    nc.tensor.matmul(
                        ps[j][:, c * P : (c + 1) * P],
                        spk_t[:, si_rel * 32 : (si_rel + 1) * 32],
                        at[:, kl * P : (kl + 1) * P],
                        start=(k == first_k[ti]),
                        stop=(k == last_k[ti]),
                    )
                    if k == last_k[ti] and ti % NJ == NJ - 1:
                        # Bank j fully accumulated: drain it while the
                        # remaining banks' matmuls keep streaming. The store
                        # goes on the ACT HWDGE ring — on the (in-order) SP
                        # ring its semaphore wait would block later block-
                        # group DMA issues behind it.
                        sl = slice(j * NJ * P, (j + 1) * NJ * P)
                        nc.vector.tensor_copy(ot[:, sl], ps[j][:, :])
                        nc.scalar.dma_start(out[:, sl], ot[:, sl])
                    k += 1

    nc.compile()
    return nc


def _group_sizes(n_blocks):
    """DMA group sizes: 1 MiB groups, but taper the tail so the last
    group's matmuls + completion latency (critical path) are short."""
    sizes = []
    rem = n_blocks
    while rem > BLK_GROUP:
        sizes.append(BLK_GROUP)
        rem -= BLK_GROUP
    while rem > 4:
        h = max(4, rem // 2)
        sizes.append(h)
        rem -= h
    if rem:
        sizes.append(rem)
    return sizes


def _get_nc(pattern, n_spk):
    key = (tuple(pattern), n_spk)
    if key not in _cache:
        _cache[key] = _build_nc(pattern, n_spk)
    return _cache[key]


def _split_hi_lo(x32):
    """Split fp32 array into (hi, lo) fp16 parts with x32 ~= hi + lo."""
    hi = x32.astype(np.float16)
    lo = (x32 - hi.astype(np.float32)).astype(np.float16)
    return hi, lo


def _prep_inputs(spikes, adjacency):
    flat = np.ascontiguousarray(np.asarray(spikes, dtype=np.float32).reshape(B, N))
    adj = np.asarray(adjacency, dtype=np.float32)

    # Live [ti, si] block map per core: block contributes to core i's
    # outputs iff adj[i*TSH + ti*128 : .. + 128, si*128 : (si+1)*128] has a
    # nonzero. Shipping exactly the live blocks keeps the kernel exact for
    # every input while skipping the zero blocks of conv-structured
    # adjacencies.
    bm = np.any(
        adj.reshape(NCORES, TI, W, SCHUNKS, P) != 0.0, axis=(2, 4)
    )  # [core, ti, si]

    # Align per-core block sets into one shared pattern via a per-core
    # source offset o_i (cores run one SPMD program). o_i = min(si - ti)
    # over live blocks aligns banded structures exactly.
    offs = np.zeros(NCORES, np.int64)
    pat = set()
    for i in range(NCORES):
        tis, sis = np.nonzero(bm[i])
        offs[i] = (sis - tis).min() if len(tis) else 0
        pat.update(zip(tis.tolist(), (sis - offs[i]).tolist()))
    for ti in range(TI):  # every ti needs >=1 block so PSUM gets initialized
        if not any(t == ti for t, _ in pat):
            pat.add((ti, 0))
    pattern = sorted(pat)
    n_spk = max(s for _, s in pattern) + 1

    # Stationary weights (hi/lo split), indexed by absolute source chunk.
    flatT = np.ascontiguousarray(flat.T)  # [N, B]
    fhi, flo = _split_hi_lo(flatT)
    spk_full = np.zeros((SCHUNKS, P, 32), np.float16)  # [si, p, 2*B]
    spk_full[:, :, :B] = fhi.reshape(SCHUNKS, P, B)
    spk_full[:, :, B:] = flo.reshape(SCHUNKS, P, B)

    n_blocks = len(pattern)
    group_sizes = _group_sizes(n_blocks)

    in_maps = []
    for i in range(NCORES):
        o = int(offs[i])
        t0 = i * TSH
        blocks = np.zeros((n_blocks, P, P), np.float16)  # [k, sj, tj]
        for k, (ti, si_rel) in enumerate(pattern):
            si = o + si_rel
            if 0 <= si < SCHUNKS and bm[i, ti, si]:
                blocks[k] = (
                    adj[t0 + ti * W : t0 + (ti + 1) * W, si * P : (si + 1) * P]
                    .T.astype(np.float16)
                )
        # Pack each DMA group partition-major: [p, gsz*128].
        parts = []
        k0 = 0
        for gsz in group_sizes:
            parts.append(
                np.ascontiguousarray(blocks[k0 : k0 + gsz].transpose(1, 0, 2)).ravel()
            )
            k0 += gsz
        ablk = np.concatenate(parts)

        spk = np.zeros((n_spk, P, 32), np.float16)
        s_lo = max(0, -o)
        s_hi = min(n_spk, SCHUNKS - o)
        if s_hi > s_lo:
            spk[s_lo:s_hi] = spk_full[o + s_lo : o + s_hi]
        spk = np.ascontiguousarray(spk.transpose(1, 0, 2)).reshape(P, n_spk * 32)
        in_maps.append({"ablk": ablk, "spk": spk})
    return pattern, n_spk, in_maps


def _run(pattern, n_spk, in_maps, **kwargs):
    from concourse.bass_utils import run_bass_kernel_spmd

    return run_bass_kernel_spmd(
        _get_nc(pattern, n_spk), in_maps, core_ids=list(range(NCORES)), **kwargs
    )


def kernel(spikes, adjacency):
    pattern, n_spk, in_maps = _prep_inputs(spikes, adjacency)
    res = _run(pattern, n_spk, in_maps)
    outs = [r["o"] for r in res.results]
    # Fold hi-weight rows (0:16) + lo-weight rows (16:32), concat target shards.
    full = np.concatenate([o[:B] + o[B:] for o in outs], axis=1)  # [B, N]
    return np.ascontiguousarray(full.reshape(B, H, W), dtype=np.float32)


# revision 36
# speedup vs baseline: 1.9616x; 1.0380x over previous
# Trainium2 Bass kernel for nn_AxonalConnections (gnn_message_passing).
#
# Computes out[B, H, W] = (spikes.reshape(B, N) @ adjacency.T).reshape(B, H, W)
# with B=16, H=W=128, N=16384 on 8 NeuronCores.
#
# Strategy (pure tensor parallelism, no collectives):
#   - Shard adjacency row-wise (target dim) across 8 cores: core i owns
#     target columns [i*2048, (i+1)*2048) of the output.
#   - The kernel is HBM/DMA-bandwidth bound, so minimize shipped bytes:
#     * adjacency ships as fp16 (values ~N(0, 0.02^2), well inside fp16
#       range; 2^-11 relative representation error -> ~1e-4 output error).
#       fp32 would also stream 4x slower through the PE; fp16 streams at
#       full rate (1 column/cycle).
#     * input-adaptive block pruning: the host scans the adjacency at
#       [128 x 128] block granularity (source grid-row si x target grid-row
#       ti) and ships only blocks that contain nonzeros. For the conv-
#       structured adjacency this is ~112 of 2048 blocks per core (3.7 MiB
#       vs 64 MiB); for dense inputs every block ships and the kernel stays
#       exact. Per-core block sets are aligned by a per-core source offset
#       into one shared pattern so all 8 cores run the same NEFF.
#   - Spikes (tiny) are split into fp16 hi + fp16 lo (exact to ~2^-22) and
#     packed as the stationary operand [spikes_hi | spikes_lo] (32 columns).
#     PSUM accumulates [32, 128]-per-block into 4 banks of [32, 512]; rows
#     0-15 = hi terms, rows 16-31 = lo terms; host folds and concatenates.
#   - Blocks stream ti-major, so each PSUM bank finishes early and its
#     PSUM->SBUF copy + output DMA overlap the remaining matmuls.
#
# Single-queue HWDGE DMA with 8 KiB per-partition runs sustains ~410 GB/s
# (95% of the 435 GB/s SBUF-AXI fabric ceiling).

import numpy as np

B = 16
H = 128
W = 128
N = H * W            # 16384 source == target size
NCORES = 8
TSH = N // NCORES    # 2048 target columns per core
TI = TSH // W        # 16 target grid-rows per core
P = 128              # SBUF partitions / contraction tile
SCHUNKS = N // P     # 128 source chunks (== source grid-rows)
BLK_GROUP = 32       # blocks per DMA (32 * 32 KiB = 1 MiB, 8 KiB runs)
BLK = P * P          # elements per block

_cache = {}


N_WARM = 18  # PE warmup matmuls (~5 us of dummy work releases the HAM clock gate)


def _plan_segments(pattern, group_sizes):
    """Plan merged matmuls over the si-major block stream.

    pattern: list of (ti, si_rel), si-major then ti-ascending — the stream
    order. Blocks with consecutive ti, the same source chunk, the same PSUM
    bank, and the same DMA group merge into one matmul of N = 128*len.

    start=True is set ONLY on the first segment of each PSUM bank: on HW it
    clears has_written for the WHOLE bank, and the per-element has_written
    bit then makes every region's first write an overwrite and later writes
    accumulates — no per-region start flags needed (a later start=True
    would wipe the has_written state of sibling regions mid-accumulation).

    Returns segments: list of (k0, nblk, si_rel, ti0, start).
    """
    group_of = []
    for g, gsz in enumerate(group_sizes):
        group_of += [g] * gsz
    segments = []
    k = 0
    n = len(pattern)
    seen_banks = set()
    while k < n:
        ti0, s = pattern[k]
        ln = 1
        while (
            k + ln < n
            and pattern[k + ln] == (ti0 + ln, s)
            and (ti0 + ln) // 4 == ti0 // 4
            and group_of[k + ln] == group_of[k]
        ):
            ln += 1
        bank = ti0 // 4
        segments.append((k, ln, s, ti0, bank not in seen_banks))
        seen_banks.add(bank)
        k += ln
    return segments


def _build_nc(pattern, n_spk):
    """Build + compile the SPMD Bass program.

    pattern: list of (ti, si_rel) block coordinates in si-major stream
             order, identical for all cores. Every ti in [0, TI) appears.
    n_spk:   number of stationary source chunks shipped (max si_rel + 1).
    """
    import concourse.mybir as mybir
    import concourse.tile as tile
    from concourse import bacc

    n_blocks = len(pattern)
    group_sizes = _group_sizes(n_blocks)
    segments = _plan_segments(pattern, group_sizes)

    nc = bacc.Bacc(
        "TRN2",
        target_bir_lowering=False,
        debug=False,
        num_devices=NCORES,
    )
    # ablk: flat stream of gathered [128 x 128] fp16 blocks in `pattern`
    # order, packed per DMA-group as [p, group_blocks*128] (partition-major)
    # so every descriptor moves one contiguous 8 KiB run per partition.
    ablk = nc.dram_tensor(
        "ablk", [n_blocks * BLK], mybir.dt.float16, kind="ExternalInput"
    ).ap()
    # spk: stationary weights for the shipped source-chunk window, packed
    # [P, n_spk*32] fp16 where
    #   spk[p, k*32 + b]      = fp16_hi(spikes[b, (o_i + k)*128 + p])
    #   spk[p, k*32 + 16 + b] = fp16_lo(spikes[b, (o_i + k)*128 + p])
    # (o_i = per-core source offset; out-of-range chunks are zero).
    spk = nc.dram_tensor(
        "spk", [P, n_spk * 32], mybir.dt.float16, kind="ExternalInput"
    ).ap()
    out = nc.dram_tensor("o", [32, TSH], mybir.dt.float32, kind="ExternalOutput").ap()

    f32 = mybir.dt.float32
    f16 = mybir.dt.float16
    NJ = 4  # PSUM banks ([32, 512] each; 4 ti-blocks per bank)

    # Last stream index per PSUM bank (drives the drain copies).
    last_k_bank = {}
    for k, (ti, _) in enumerate(pattern):
        last_k_bank[ti // NJ] = k

    # Map stream index -> (group, local index, group start offset).
    grp_of = []
    for g, gsz in enumerate(group_sizes):
        base = len(grp_of)
        grp_of += [(g, kk - base) for kk in range(base, base + gsz)]

    with tile.TileContext(nc) as tc:
        with (
            tc.tile_pool(name="adj", bufs=min(8, len(group_sizes))) as adj_pool,
            tc.tile_pool(name="spkp", bufs=1) as spk_pool,
            tc.tile_pool(name="warm", bufs=1) as warm_pool,
            tc.tile_pool(name="psum", bufs=1, space="PSUM") as psum_pool,
            tc.tile_pool(name="outp", bufs=1) as out_pool,
        ):
            # PE warmup: ~5 us of dummy matmuls on a zeroed tile, scheduled
            # before any real data arrives. They release the HAM clock gate
            # (cold PE runs at 1.2 GHz for the first ~3.4 us of activity) so
            # the real matmuls run at 2.4 GHz from the start.
            dumt = warm_pool.tile([P, 512], f16)
            nc.gpsimd.memset(dumt[:], 0.0)
            psw = psum_pool.tile([32, 512], f32, name="psw", tag="psw")
            for _ in range(N_WARM):
                nc.tensor.matmul(
                    psw[:, :],
                    dumt[:, 0:32],
                    dumt[:, :],
                    start=True,
                    stop=True,
                    skip_group_check=True,
                )

            # Stationary weights load first on the SP ring: every matmul
            # waits on them, and on the ACT ring their packets get
            # interleaved behind the block stream (first matmul slips by
            # ~2.5 us). Serializing ~0.5 us ahead of the stream is cheaper.
            spk_t = spk_pool.tile([P, n_spk * 32], f16)
            nc.sync.dma_start(spk_t[:], spk[:])

            ps = [
                psum_pool.tile([32, NJ * P], f32, name=f"ps{j}", tag=f"ps{j}")
                for j in range(NJ)
            ]
            ot = out_pool.tile([32, TSH], f32)

            at_tiles = []
            off = 0
            for g, gsz in enumerate(group_sizes):
                at = adj_pool.tile([P, gsz * P], f16, name=f"at{g}", tag="at")
                nc.sync.dma_start(
                    at[:].rearrange("p (n t) -> p n t", n=gsz),
                    ablk[off : off + gsz * BLK].rearrange("(p n t) -> p n t", p=P, t=P),
                )
                off += gsz * BLK
                at_tiles.append(at)

            for k0, nblk, si_rel, ti0, start in segments:
                g, kl = grp_of[k0]
                j, c = divmod(ti0, NJ)
                nc.tensor.matmul(
                    ps[j][:, c * P : (c + 1 + nblk - 1) * P],
                    spk_t[:, si_rel * 32 : (si_rel + 1) * 32],
                    at_tiles[g][:, kl * P : (kl + nblk) * P],
                    start=start,
                    stop=(k0 + nblk - 1 == last_k_bank[j]),
                    skip_group_check=True,
                )
                if k0 + nblk - 1 == last_k_bank[j]:
                    # Bank j fully accumulated: drain it while the remaining
                    # banks' matmuls keep streaming. The store goes on the
                    # ACT HWDGE ring — on the (in-order) SP ring its
                    # semaphore wait would block later DMA issues behind it.
                    sl = slice(j * NJ * P, (j + 1) * NJ * P)
                    nc.vector.tensor_copy(ot[:, sl], ps[j][:, :])
                    nc.scalar.dma_start(out[:, sl], ot[:, sl])

    nc.compile()
    return nc


def _group_sizes(n_blocks):
    """DMA group sizes: 1 MiB groups, but taper the tail so the last
    group's matmuls + completion latency (critical path) are short."""
    sizes = []
    rem = n_blocks
    while rem > BLK_GROUP:
        sizes.append(BLK_GROUP)
        rem -= BLK_GROUP
    while rem > 4:
        h = max(4, rem // 2)
        sizes.append(h)
        rem -= h
    if rem:
        sizes.append(rem)
    return sizes


def _get_nc(pattern, n_spk):
    key = (tuple(pattern), n_spk)
    if key not in _cache:
        _cache[key] = _build_nc(pattern, n_spk)
    return _cache[key]


def _split_hi_lo(x32):
    """Split fp32 array into (hi, lo) fp16 parts with x32 ~= hi + lo."""
    hi = x32.astype(np.float16)
    lo = (x32 - hi.astype(np.float32)).astype(np.float16)
    return hi, lo


def _prep_inputs(spikes, adjacency):
    flat = np.ascontiguousarray(np.asarray(spikes, dtype=np.float32).reshape(B, N))
    adj = np.asarray(adjacency, dtype=np.float32)

    # Live [ti, si] block map per core: block contributes to core i's
    # outputs iff adj[i*TSH + ti*128 : .. + 128, si*128 : (si+1)*128] has a
    # nonzero. Shipping exactly the live blocks keeps the kernel exact for
    # every input while skipping the zero blocks of conv-structured
    # adjacencies.
    bm = np.any(
        adj.reshape(NCORES, TI, W, SCHUNKS, P) != 0.0, axis=(2, 4)
    )  # [core, ti, si]

    # Align per-core block sets into one shared pattern via a per-core
    # source offset o_i (cores run one SPMD program). o_i = min(si - ti)
    # over live blocks aligns banded structures exactly.
    offs = np.zeros(NCORES, np.int64)
    pat = set()
    for i in range(NCORES):
        tis, sis = np.nonzero(bm[i])
        offs[i] = (sis - tis).min() if len(tis) else 0
        pat.update(zip(tis.tolist(), (sis - offs[i]).tolist()))
    for ti in range(TI):  # every ti needs >=1 block so PSUM gets initialized
        if not any(t == ti for t, _ in pat):
            pat.add((ti, 0))
    # si-major, ti-ascending stream order (enables merged matmuls over
    # consecutive ti sharing one stationary source chunk).
    pattern = sorted(pat, key=lambda x: (x[1], x[0]))
    n_spk = max(s for _, s in pattern) + 1

    # Stationary weights (hi/lo split), indexed by absolute source chunk.
    flatT = np.ascontiguousarray(flat.T)  # [N, B]
    fhi, flo = _split_hi_lo(flatT)
    spk_full = np.zeros((SCHUNKS, P, 32), np.float16)  # [si, p, 2*B]
    spk_full[:, :, :B] = fhi.reshape(SCHUNKS, P, B)
    spk_full[:, :, B:] = flo.reshape(SCHUNKS, P, B)

    n_blocks = len(pattern)
    group_sizes = _group_sizes(n_blocks)

    in_maps = []
    for i in range(NCORES):
        o = int(offs[i])
        t0 = i * TSH
        blocks = np.zeros((n_blocks, P, P), np.float16)  # [k, sj, tj]
        for k, (ti, si_rel) in enumerate(pattern):
            si = o + si_rel
            if 0 <= si < SCHUNKS and bm[i, ti, si]:
                blocks[k] = (
                    adj[t0 + ti * W : t0 + (ti + 1) * W, si * P : (si + 1) * P]
                    .T.astype(np.float16)
                )
        # Pack each DMA group partition-major: [p, gsz*128].
        parts = []
        k0 = 0
        for gsz in group_sizes:
            parts.append(
                np.ascontiguousarray(blocks[k0 : k0 + gsz].transpose(1, 0, 2)).ravel()
            )
            k0 += gsz
        ablk = np.concatenate(parts)

        spk = np.zeros((n_spk, P, 32), np.float16)
        s_lo = max(0, -o)
        s_hi = min(n_spk, SCHUNKS - o)
        if s_hi > s_lo:
            spk[s_lo:s_hi] = spk_full[o + s_lo : o + s_hi]
        spk = np.ascontiguousarray(spk.transpose(1, 0, 2)).reshape(P, n_spk * 32)
        in_maps.append({"ablk": ablk, "spk": spk})
    return pattern, n_spk, in_maps


def _run(pattern, n_spk, in_maps, **kwargs):
    from concourse.bass_utils import run_bass_kernel_spmd

    return run_bass_kernel_spmd(
        _get_nc(pattern, n_spk), in_maps, core_ids=list(range(NCORES)), **kwargs
    )


def kernel(spikes, adjacency):
    pattern, n_spk, in_maps = _prep_inputs(spikes, adjacency)
    res = _run(pattern, n_spk, in_maps)
    outs = [r["o"] for r in res.results]
    # Fold hi-weight rows (0:16) + lo-weight rows (16:32), concat target shards.
    full = np.concatenate([o[:B] + o[B:] for o in outs], axis=1)  # [B, N]
    return np.ascontiguousarray(full.reshape(B, H, W), dtype=np.float32)
